# revision 11
# baseline (speedup 1.0000x reference)
"""nms_detection kernel for 8 TRN2 NeuronCores.

Pipeline:
  host:    transpose conf [B,A,C] -> [B,C,A]
  device1: per-(class, 1024-chunk) top-16 selection on raw conf
           (max8 + max_index + match_replace + max8 + max_index),
           dense SSD box decode + area -> box table [A, 8]
  host:    gather table rows + exact raw conf at selected indices
  device2: 64-step greedy NMS over the 256-candidate pool per
           (batch, class) lane on raw conf (sigmoid is monotonic);
           sigmoid (XLA-matching cephes exp chain) applied to the 64
           winning scores only.
"""
import numpy as np
import concourse.bacc as bacc
import concourse.bass as bass
import concourse.mybir as mybir
import concourse.tile as tile
from concourse.bass_utils import run_bass_kernel_spmd

f32 = mybir.dt.float32
i32 = mybir.dt.int32
u32 = mybir.dt.uint32
Alu = mybir.AluOpType

B, A, C = 16, 16384, 81
K = 64                 # TOP_K
CH = 1024              # selection chunk size
NCH = A // CH          # 16 chunks
W = NCH * 16           # pool = top-16 per chunk = 256
NCORES = 8
BPC = B // NCORES      # batches per core
PA = A // 128          # anchors per partition in natural layout

NEG = -1.0e30          # masked/suppressed sentinel
# sigmoid(conf) > 0.3  <=>  conf > XSTAR (verified on the data; 8 ulp margin)
XSTAR = float(np.float32(-0.84729767))

# cephes/XLA-CPU expf constants
LOG2E = float(np.float32(1.44269504088896341))
EC1 = float(np.float32(0.693359375))
EC2 = float(np.float32(-2.12194440e-4))
EP = [float(np.float32(v)) for v in (1.9875691500e-4, 1.3981999507e-3,
                                     8.3334519073e-3, 4.1665795894e-2,
                                     1.6666665459e-1, 5.0000001201e-1)]


def _exp_chain(nc, pool, x, P, N, tagp, eng=None):
    """exp(x) replicating XLA-CPU expf (cephes, no-FMA variant).
    x: SBUF AP [P, N] f32. Returns a [P, N] f32 tile."""
    if eng is None:
        eng = nc.vector
    m = pool.tile([P, N], f32, tag=tagp + "m")
    t_i = pool.tile([P, N], i32, tag=tagp + "ti")
    tf = pool.tile([P, N], f32, tag=tagp + "tf")
    r = pool.tile([P, N], f32, tag=tagp + "r")
    z = pool.tile([P, N], f32, tag=tagp + "z")
    y = pool.tile([P, N], f32, tag=tagp + "y")
    s1 = pool.tile([P, N], f32, tag=tagp + "s1")
    out = pool.tile([P, N], f32, tag=tagp + "o")
    # m = floor(x*LOG2E + 0.5)
    eng.tensor_scalar(m, x, LOG2E, 0.5, Alu.mult, Alu.add)
    eng.tensor_copy(t_i, m)
    eng.tensor_copy(tf, t_i)
    eng.tensor_tensor(out=s1, in0=tf, in1=m, op=Alu.is_gt)
    eng.tensor_tensor(out=m, in0=tf, in1=s1, op=Alu.subtract)
    # r = (x - m*C1) - m*C2
    eng.tensor_scalar(s1, m, EC1, None, Alu.mult)
    eng.tensor_tensor(out=r, in0=x, in1=s1, op=Alu.subtract)
    eng.tensor_scalar(s1, m, EC2, None, Alu.mult)
    eng.tensor_tensor(out=r, in0=r, in1=s1, op=Alu.subtract)
    eng.tensor_tensor(out=z, in0=r, in1=r, op=Alu.mult)
    # Horner
    eng.tensor_scalar(y, r, EP[0], EP[1], Alu.mult, Alu.add)
    for p in EP[2:]:
        eng.tensor_tensor(out=y, in0=y, in1=r, op=Alu.mult)
        eng.tensor_scalar(y, y, p, None, Alu.add)
    eng.tensor_tensor(out=y, in0=y, in1=z, op=Alu.mult)
    eng.tensor_tensor(out=y, in0=y, in1=r, op=Alu.add)
    eng.tensor_scalar(y, y, 1.0, None, Alu.add)
    # 2^m: (int(m)+127) << 23 bitcast to f32
    eng.tensor_copy(t_i, m)
    eng.tensor_scalar(t_i, t_i, 127, None, Alu.add)
    eng.tensor_scalar(t_i, t_i, 23, None, Alu.logical_shift_left)
    eng.tensor_tensor(out=out, in0=y, in1=t_i.bitcast(f32), op=Alu.mult)
    return out


def _build_launch1():
    nc = bacc.Bacc(None, target_bir_lowering=False)
    with tile.TileContext(nc) as tc:
        with tc.tile_pool(name="dram", bufs=1, space="DRAM") as dram, \
             tc.tile_pool(name="sb", bufs=1) as pool:
            confT = dram.tile([BPC, C, A], f32, kind="ExternalInput")
            locd = dram.tile([BPC, A, 4], f32, kind="ExternalInput")
            anch = dram.tile([A, 4], f32, kind="ExternalInput")
            pi_out = dram.tile([BPC, C, W], u32, kind="ExternalOutput")
            tab_out = dram.tile([BPC, A, 8], f32, kind="ExternalOutput")

            an = pool.tile([128, PA, 4], f32)
            nc.sync.dma_start(out=an,
                              in_=anch[:, :].rearrange("(p k) f -> p k f", p=128))
            ioff = pool.tile([C, NCH, 16], u32)
            nc.gpsimd.iota(ioff, pattern=[[CH, NCH], [0, 16]], base=0,
                           channel_multiplier=0)

            for b in range(BPC):
                # ---- top-16 per 1024-chunk on raw conf ----
                ct = pool.tile([C, A], f32, tag=f"ct{b}")
                nc.sync.dma_start(out=ct, in_=confT[b, :, :])
                micat = pool.tile([C, NCH, 16], u32, tag=f"mi{b}")
                mv1 = pool.tile([C, 8], f32, tag=f"mv1{b}")
                mv2 = pool.tile([C, 8], f32, tag=f"mv2{b}")
                for ch in range(NCH):
                    sl = ct[:, ch * CH:(ch + 1) * CH]
                    nc.vector.max(out=mv1, in_=sl)
                    nc.vector.max_index(out=micat[:, ch, 0:8], in_max=mv1,
                                        in_values=sl)
                    nc.vector.match_replace(out=sl, in_to_replace=mv1,
                                            in_values=sl, imm_value=NEG)
                    nc.vector.max(out=mv2, in_=sl)
                    nc.vector.max_index(out=micat[:, ch, 8:16], in_max=mv2,
                                        in_values=sl)
                gi = pool.tile([C, NCH, 16], u32, tag=f"gi{b}")
                nc.vector.tensor_tensor(out=gi, in0=micat, in1=ioff, op=Alu.add)
                nc.sync.dma_start(
                    out=pi_out[b, :, :].rearrange("c (n e) -> c n e", e=16),
                    in_=gi)

                # ---- dense decode ----
                lo = pool.tile([128, PA, 4], f32, tag=f"lo{b}")
                nc.sync.dma_start(out=lo,
                                  in_=locd[b, :, :].rearrange("(p k) f -> p k f", p=128))
                tabt = pool.tile([128, PA, 8], f32, tag=f"tabt{b}")
                ein = pool.tile([128, PA * 2], f32, tag=f"ein{b}")
                nc.vector.tensor_scalar(
                    ein[:, :].rearrange("p (k f) -> p k f", f=2),
                    lo[:, :, 2:4], 0.2, None, Alu.mult)
                ex = _exp_chain(nc, pool, ein[:, :], 128, PA * 2, f"e{b}")
                wh = pool.tile([128, PA, 2], f32, tag=f"wh{b}")
                nc.vector.tensor_tensor(
                    out=wh, in0=an[:, :, 2:4],
                    in1=ex[:, :].rearrange("p (k f) -> p k f", f=2), op=Alu.mult)
                t0 = pool.tile([128, PA, 2], f32, tag=f"t0{b}")
                nc.vector.tensor_scalar(t0, lo[:, :, 0:2], 0.1, None, Alu.mult)
                nc.vector.tensor_tensor(out=t0, in0=t0, in1=an[:, :, 2:4], op=Alu.mult)
                nc.vector.tensor_tensor(out=t0, in0=t0, in1=an[:, :, 0:2], op=Alu.add)
                t1 = pool.tile([128, PA, 2], f32, tag=f"t1{b}")
                nc.vector.tensor_scalar(t1, wh, 0.5, None, Alu.mult)
                nc.vector.tensor_tensor(out=tabt[:, :, 0:2], in0=t0, in1=t1,
                                        op=Alu.subtract)
                nc.vector.tensor_tensor(out=tabt[:, :, 2:4], in0=tabt[:, :, 0:2],
                                        in1=wh, op=Alu.add)
                t2 = pool.tile([128, PA, 2], f32, tag=f"t2{b}")
                nc.vector.tensor_tensor(out=t2, in0=tabt[:, :, 2:4],
                                        in1=tabt[:, :, 0:2], op=Alu.subtract)
                nc.vector.tensor_tensor(out=tabt[:, :, 4:5], in0=t2[:, :, 0:1],
                                        in1=t2[:, :, 1:2], op=Alu.mult)
                nc.vector.memset(tabt[:, :, 5:8], 0.0)
                nc.sync.dma_start(
                    out=tab_out[b, :, :].rearrange("(p k) f -> p k f", p=128),
                    in_=tabt)
    nc.compile()
    names = dict(confT=confT.name, locd=locd.name, anch=anch.name,
                 pi=pi_out.name, tab=tab_out.name)
    return nc, names


def _build_launch2(steps=K):
    nc = bacc.Bacc(None, target_bir_lowering=False)
    with tile.TileContext(nc) as tc:
        with tc.tile_pool(name="dram", bufs=1, space="DRAM") as dram, \
             tc.tile_pool(name="sb", bufs=1) as pool:
            g_in = dram.tile([BPC, C, 5, W], f32, kind="ExternalInput")
            pv_in = dram.tile([BPC, C, W], f32, kind="ExternalInput")
            rows_out = dram.tile([BPC, C, K, 8], f32, kind="ExternalOutput")

            iot = pool.tile([C, W], f32)
            nc.gpsimd.iota(iot, pattern=[[1, W]], base=0, channel_multiplier=0,
                           allow_small_or_imprecise_dtypes=True)
            negC = pool.tile([C, W], f32)
            nc.vector.memset(negC, NEG)

            for b in range(BPC):
                # batch 0's NMS chain runs on DVE; batch 1's elementwise chain
                # runs on the Pool engine (max/max_index are DVE-only)
                eng = nc.vector if b == 0 else nc.gpsimd
                G = pool.tile([C, 5, W], f32, tag=f"G{b}")
                nc.sync.dma_start(out=G, in_=g_in[b, :, :, :])
                pv = pool.tile([C, W], f32, tag=f"pv{b}")
                nc.sync.dma_start(out=pv, in_=pv_in[b, :, :])

                # s = where(conf > x*, conf, NEG) on exact raw conf
                cmp = pool.tile([C, W], u32, tag=f"cmp{b}")
                s = pool.tile([C, W], f32, tag=f"s{b}")
                nc.vector.tensor_scalar(cmp, pv, XSTAR, None, Alu.is_gt)
                nc.vector.select(out=s, mask=cmp, on_true=pv, on_false=negC)

                outb = pool.tile([C, K, 8], f32, tag=f"outb{b}")
                if b != 0:
                    nc.vector.memset(outb[:, :, 6:8], 0.0)

                m8 = pool.tile([C, 8], f32, tag=f"m8{b}")
                i8 = pool.tile([C, 8], u32, tag=f"i8{b}")
                jf = pool.tile([C, 1], f32, tag=f"jf{b}")
                scr = pool.tile([C, W], f32, tag=f"scr{b}")
                eqf = pool.tile([C, W], f32, tag=f"eqf{b}")
                wh2 = pool.tile([C, 2], f32, tag=f"wh2{b}")
                m1 = pool.tile([C, W], f32, tag=f"m1{b}")
                m2 = pool.tile([C, W], f32, tag=f"m2{b}")
                wx = pool.tile([C, W], f32, tag=f"wx{b}")
                wy = pool.tile([C, W], f32, tag=f"wy{b}")
                cx = pool.tile([C, W], f32, tag=f"cx{b}")
                cy = pool.tile([C, W], f32, tag=f"cy{b}")
                inter = pool.tile([C, W], f32, tag=f"int{b}")
                un = pool.tile([C, W], f32, tag=f"un{b}")
                ddm = pool.tile([C, W], f32, tag=f"ddm{b}")
                V = nc.vector
                stt = V.scalar_tensor_tensor
                for t in range(steps):
                    if b == 0:
                        # === all-DVE chain, stt-fused (15 W-passes) ===
                        # max writes the row head directly: field 0 = score
                        V.max(out=outb[:, t, 0:8], in_=s)
                        V.max_index(out=i8, in_max=outb[:, t, 0:8], in_values=s)
                        V.tensor_copy(jf, i8[:, 0:1])
                        # box extraction: coords -> outb[:, t, 1:5]
                        for f in range(4):
                            stt(out=scr, in0=iot, scalar=jf[:, 0:1],
                                in1=G[:, f, :], op0=Alu.is_equal, op1=Alu.mult,
                                accum_out=outb[:, t, f + 1:f + 2])
                        # selected area from corners (reference fp-op order)
                        V.tensor_tensor(out=wh2, in0=outb[:, t, 3:5],
                                        in1=outb[:, t, 1:3], op=Alu.subtract)
                        V.tensor_tensor(out=outb[:, t, 5:6], in0=wh2[:, 0:1],
                                        in1=wh2[:, 1:2], op=Alu.mult)
                        # IoU: w = min(Gx2,X2)-max(Gx1,X1), same for y
                        V.tensor_scalar(m1, G[:, 0, :], outb[:, t, 1:2],
                                        None, Alu.max)
                        stt(out=wx, in0=G[:, 2, :], scalar=outb[:, t, 3:4],
                            in1=m1, op0=Alu.min, op1=Alu.subtract)
                        V.tensor_scalar(m1, G[:, 1, :], outb[:, t, 2:3],
                                        None, Alu.max)
                        stt(out=wy, in0=G[:, 3, :], scalar=outb[:, t, 4:5],
                            in1=m1, op0=Alu.min, op1=Alu.subtract)
                        V.tensor_scalar(cx, wx, 0.0, None, Alu.max)
                        stt(out=inter, in0=wy, scalar=0.0, in1=cx,
                            op0=Alu.max, op1=Alu.mult)
                        # union = (a_j + a_sel) - inter
                        stt(out=un, in0=G[:, 4, :], scalar=outb[:, t, 5:6],
                            in1=inter, op0=Alu.add, op1=Alu.subtract)
                        # suppress iff inter > 0.5*union (no boundary flips)
                        stt(out=ddm, in0=un, scalar=0.5, in1=inter,
                            op0=Alu.mult, op1=Alu.is_lt)
                        # s -= 1e30 * ddm  (== where(ddm, NEG, s))
                        stt(out=s, in0=ddm, scalar=NEG, in1=s,
                            op0=Alu.mult, op1=Alu.add)
                    else:
                        # === DVE/Pool split: Pool has no stt/compare/reduce,
                        # so DVE keeps max/index/extract/compare/suppress and
                        # Pool runs the pure-arithmetic IoU chain ===
                        P = nc.gpsimd
                        V.max(out=m8, in_=s)
                        V.max_index(out=i8, in_max=m8, in_values=s)
                        V.tensor_copy(outb[:, t, 0:1], m8[:, 0:1])
                        V.tensor_copy(jf, i8[:, 0:1])
                        for f in range(4):
                            stt(out=scr, in0=iot, scalar=jf[:, 0:1],
                                in1=G[:, f, :], op0=Alu.is_equal, op1=Alu.mult,
                                accum_out=outb[:, t, f + 1:f + 2])
                        V.tensor_tensor(out=wh2, in0=outb[:, t, 3:5],
                                        in1=outb[:, t, 1:3], op=Alu.subtract)
                        V.tensor_tensor(out=outb[:, t, 5:6], in0=wh2[:, 0:1],
                                        in1=wh2[:, 1:2], op=Alu.mult)
                        P.tensor_scalar(m1, G[:, 0, :], outb[:, t, 1:2],
                                        None, Alu.max)
                        P.tensor_scalar(m2, G[:, 2, :], outb[:, t, 3:4],
                                        None, Alu.min)
                        P.tensor_tensor(out=wx, in0=m2, in1=m1, op=Alu.subtract)
                        P.tensor_scalar(m1, G[:, 1, :], outb[:, t, 2:3],
                                        None, Alu.max)
                        P.tensor_scalar(m2, G[:, 3, :], outb[:, t, 4:5],
                                        None, Alu.min)
                        P.tensor_tensor(out=wy, in0=m2, in1=m1, op=Alu.subtract)
                        P.tensor_scalar(cx, wx, 0.0, None, Alu.max)
                        P.tensor_scalar(cy, wy, 0.0, None, Alu.max)
                        P.tensor_tensor(out=inter, in0=cx, in1=cy, op=Alu.mult)
                        P.tensor_scalar(un, G[:, 4, :], outb[:, t, 5:6],
                                        None, Alu.add)
                        P.tensor_tensor(out=un, in0=un, in1=inter, op=Alu.subtract)
                        P.tensor_scalar(ddm, un, 0.5, None, Alu.mult)
                        V.tensor_tensor(out=ddm, in0=ddm, in1=inter, op=Alu.is_lt)
                        stt(out=s, in0=ddm, scalar=NEG, in1=s,
                            op0=Alu.mult, op1=Alu.add)

                # epilogue: km mask, sigmoid on winning raw scores
                km = pool.tile([C, K], f32, tag=f"km{b}")
                xs = pool.tile([C, K], f32, tag=f"xs{b}")
                nc.vector.tensor_scalar(km, outb[:, :, 0], -1e29, None, Alu.is_gt)
                nc.vector.tensor_scalar(xs, outb[:, :, 0], -30.0, None, Alu.max)
                nc.vector.tensor_scalar(xs, xs, -1.0, None, Alu.mult)
                e = _exp_chain(nc, pool, xs[:, :], C, K, f"se{b}")
                den = pool.tile([C, K], f32, tag=f"den{b}")
                nc.vector.tensor_scalar(den, e, 1.0, None, Alu.add)
                sg = pool.tile([C, K], f32, tag=f"sg{b}")
                nc.vector.reciprocal(sg, den)
                nc.vector.tensor_copy(outb[:, :, 0], sg)
                # zero dead rows: fields 0:6 *= km
                km_ap = km[:, :]
                km_b = bass.AP(km_ap.tensor, km_ap.offset,
                               [list(km_ap.ap[0]), list(km_ap.ap[1]), [0, 6]])
                nc.vector.tensor_tensor(out=outb[:, :, 0:6], in0=outb[:, :, 0:6],
                                        in1=km_b, op=Alu.mult)
                nc.sync.dma_start(out=rows_out[b, :, :, :], in_=outb)
    nc.compile()
    names = dict(g=g_in.name, pv=pv_in.name, rows=rows_out.name)
    return nc, names


_cache = {}


def _prep_launch2_inputs(r1, n1, confT, loc=None):
    """Host gather: exact conf + box-table fields at pool indices."""
    in_maps2 = []
    for c in range(NCORES):
        res = r1.results[c]
        pi, tab = res[n1["pi"]], res[n1["tab"]]
        G = np.empty((BPC, C, 5, W), np.float32)
        pv = np.empty((BPC, C, W), np.float32)
        for b in range(BPC):
            idx = pi[b].astype(np.int64)                 # [C, W]
            G[b] = tab[b][idx][..., :5].transpose(0, 2, 1)
            pv[b] = np.take_along_axis(confT[c * BPC + b], idx, axis=1)
        in_maps2.append({_cache["n2"]["g"]: np.ascontiguousarray(G),
                         _cache["n2"]["pv"]: pv})
    return in_maps2


def kernel(loc, conf, anchors):
    loc = np.ascontiguousarray(np.asarray(loc, np.float32))
    anchors = np.ascontiguousarray(np.asarray(anchors, np.float32))
    confT = np.ascontiguousarray(np.swapaxes(np.asarray(conf, np.float32), 1, 2))

    if "l1" not in _cache:
        _cache["l1"] = _build_launch1()
        _cache["l2"] = _build_launch2()
        _cache["n1"] = _cache["l1"][1]
        _cache["n2"] = _cache["l2"][1]
    nc1, n1 = _cache["l1"]
    nc2, n2 = _cache["l2"]

    in_maps = []
    for c in range(NCORES):
        sl = slice(c * BPC, (c + 1) * BPC)
        in_maps.append({n1["confT"]: confT[sl], n1["locd"]: loc[sl],
                        n1["anch"]: anchors})
    r1 = run_bass_kernel_spmd(nc1, in_maps, core_ids=list(range(NCORES)))

    in_maps2 = _prep_launch2_inputs(r1, n1, confT)
    r2 = run_bass_kernel_spmd(nc2, in_maps2, core_ids=list(range(NCORES)))

    out = np.empty((B, C, K, 5), np.float32)
    for c in range(NCORES):
        rows = r2.results[c][n2["rows"]]
        out[c * BPC:(c + 1) * BPC] = rows[..., :5]
    return out


# revision 13
# speedup vs baseline: 2.8352x; 2.8352x over previous
"""nms_detection kernel for 8 TRN2 NeuronCores.

Pipeline:
  host:    transpose conf [B,A,C] -> [B,C,A]
  device1: per-(class, 1024-chunk) top-16 selection on raw conf
           (max8 + max_index + match_replace + max8 + max_index),
           dense SSD box decode + area -> box table [A, 8]
  host:    gather table rows + exact raw conf at selected indices
  device2: 64-step greedy NMS over the 256-candidate pool per
           (batch, class) lane on raw conf (sigmoid is monotonic);
           sigmoid (XLA-matching cephes exp chain) applied to the 64
           winning scores only.
"""
import numpy as np
import concourse.bacc as bacc
import concourse.bass as bass
import concourse.mybir as mybir
import concourse.tile as tile
from concourse.bass_utils import run_bass_kernel_spmd

f32 = mybir.dt.float32
i32 = mybir.dt.int32
u32 = mybir.dt.uint32
Alu = mybir.AluOpType

B, A, C = 16, 16384, 81
K = 64                 # TOP_K
CH = 1024              # selection chunk size
NCH = A // CH          # 16 chunks
W = NCH * 16           # pool = top-16 per chunk = 256
NCORES = 8
BPC = B // NCORES      # batches per core
PA = A // 128          # anchors per partition in natural layout

NEG = -1.0e30          # masked/suppressed sentinel
# sigmoid(conf) > 0.3  <=>  conf > XSTAR (verified on the data; 8 ulp margin)
XSTAR = float(np.float32(-0.84729767))

# cephes/XLA-CPU expf constants
LOG2E = float(np.float32(1.44269504088896341))
EC1 = float(np.float32(0.693359375))
EC2 = float(np.float32(-2.12194440e-4))
EP = [float(np.float32(v)) for v in (1.9875691500e-4, 1.3981999507e-3,
                                     8.3334519073e-3, 4.1665795894e-2,
                                     1.6666665459e-1, 5.0000001201e-1)]


def _exp_chain(nc, pool, x, P, N, tagp, eng=None):
    """exp(x) replicating XLA-CPU expf (cephes, no-FMA variant).
    x: SBUF AP [P, N] f32. Returns a [P, N] f32 tile."""
    if eng is None:
        eng = nc.vector
    m = pool.tile([P, N], f32, tag=tagp + "m")
    t_i = pool.tile([P, N], i32, tag=tagp + "ti")
    tf = pool.tile([P, N], f32, tag=tagp + "tf")
    r = pool.tile([P, N], f32, tag=tagp + "r")
    z = pool.tile([P, N], f32, tag=tagp + "z")
    y = pool.tile([P, N], f32, tag=tagp + "y")
    s1 = pool.tile([P, N], f32, tag=tagp + "s1")
    out = pool.tile([P, N], f32, tag=tagp + "o")
    # m = floor(x*LOG2E + 0.5)
    eng.tensor_scalar(m, x, LOG2E, 0.5, Alu.mult, Alu.add)
    eng.tensor_copy(t_i, m)
    eng.tensor_copy(tf, t_i)
    eng.tensor_tensor(out=s1, in0=tf, in1=m, op=Alu.is_gt)
    eng.tensor_tensor(out=m, in0=tf, in1=s1, op=Alu.subtract)
    # r = (x - m*C1) - m*C2
    eng.tensor_scalar(s1, m, EC1, None, Alu.mult)
    eng.tensor_tensor(out=r, in0=x, in1=s1, op=Alu.subtract)
    eng.tensor_scalar(s1, m, EC2, None, Alu.mult)
    eng.tensor_tensor(out=r, in0=r, in1=s1, op=Alu.subtract)
    eng.tensor_tensor(out=z, in0=r, in1=r, op=Alu.mult)
    # Horner
    eng.tensor_scalar(y, r, EP[0], EP[1], Alu.mult, Alu.add)
    for p in EP[2:]:
        eng.tensor_tensor(out=y, in0=y, in1=r, op=Alu.mult)
        eng.tensor_scalar(y, y, p, None, Alu.add)
    eng.tensor_tensor(out=y, in0=y, in1=z, op=Alu.mult)
    eng.tensor_tensor(out=y, in0=y, in1=r, op=Alu.add)
    eng.tensor_scalar(y, y, 1.0, None, Alu.add)
    # 2^m: (int(m)+127) << 23 bitcast to f32
    eng.tensor_copy(t_i, m)
    eng.tensor_scalar(t_i, t_i, 127, None, Alu.add)
    eng.tensor_scalar(t_i, t_i, 23, None, Alu.logical_shift_left)
    eng.tensor_tensor(out=out, in0=y, in1=t_i.bitcast(f32), op=Alu.mult)
    return out


def _build_launch1():
    nc = bacc.Bacc(None, target_bir_lowering=False)
    with tile.TileContext(nc) as tc:
        with tc.tile_pool(name="dram", bufs=1, space="DRAM") as dram, \
             tc.tile_pool(name="sb", bufs=1) as pool:
            confT = dram.tile([BPC, C, A], f32, kind="ExternalInput")
            locd = dram.tile([BPC, A, 4], f32, kind="ExternalInput")
            anch = dram.tile([A, 4], f32, kind="ExternalInput")
            pi_out = dram.tile([BPC, C, W], u32, kind="ExternalOutput")
            tab_out = dram.tile([BPC, A, 8], f32, kind="ExternalOutput")

            an = pool.tile([128, PA, 4], f32)
            nc.sync.dma_start(out=an,
                              in_=anch[:, :].rearrange("(p k) f -> p k f", p=128))
            ioff = pool.tile([C, NCH, 16], u32)
            nc.gpsimd.iota(ioff, pattern=[[CH, NCH], [0, 16]], base=0,
                           channel_multiplier=0)

            for b in range(BPC):
                # ---- top-16 per 1024-chunk on raw conf ----
                ct = pool.tile([C, A], f32, tag=f"ct{b}")
                nc.sync.dma_start(out=ct, in_=confT[b, :, :])
                micat = pool.tile([C, NCH, 16], u32, tag=f"mi{b}")
                mv1 = pool.tile([C, 8], f32, tag=f"mv1{b}")
                mv2 = pool.tile([C, 8], f32, tag=f"mv2{b}")
                for ch in range(NCH):
                    sl = ct[:, ch * CH:(ch + 1) * CH]
                    nc.vector.max(out=mv1, in_=sl)
                    nc.vector.max_index(out=micat[:, ch, 0:8], in_max=mv1,
                                        in_values=sl)
                    nc.vector.match_replace(out=sl, in_to_replace=mv1,
                                            in_values=sl, imm_value=NEG)
                    nc.vector.max(out=mv2, in_=sl)
                    nc.vector.max_index(out=micat[:, ch, 8:16], in_max=mv2,
                                        in_values=sl)
                gi = pool.tile([C, NCH, 16], u32, tag=f"gi{b}")
                nc.vector.tensor_tensor(out=gi, in0=micat, in1=ioff, op=Alu.add)
                nc.sync.dma_start(
                    out=pi_out[b, :, :].rearrange("c (n e) -> c n e", e=16),
                    in_=gi)

                # ---- dense decode ----
                lo = pool.tile([128, PA, 4], f32, tag=f"lo{b}")
                nc.sync.dma_start(out=lo,
                                  in_=locd[b, :, :].rearrange("(p k) f -> p k f", p=128))
                tabt = pool.tile([128, PA, 8], f32, tag=f"tabt{b}")
                ein = pool.tile([128, PA * 2], f32, tag=f"ein{b}")
                nc.vector.tensor_scalar(
                    ein[:, :].rearrange("p (k f) -> p k f", f=2),
                    lo[:, :, 2:4], 0.2, None, Alu.mult)
                ex = _exp_chain(nc, pool, ein[:, :], 128, PA * 2, f"e{b}")
                wh = pool.tile([128, PA, 2], f32, tag=f"wh{b}")
                nc.vector.tensor_tensor(
                    out=wh, in0=an[:, :, 2:4],
                    in1=ex[:, :].rearrange("p (k f) -> p k f", f=2), op=Alu.mult)
                t0 = pool.tile([128, PA, 2], f32, tag=f"t0{b}")
                nc.vector.tensor_scalar(t0, lo[:, :, 0:2], 0.1, None, Alu.mult)
                nc.vector.tensor_tensor(out=t0, in0=t0, in1=an[:, :, 2:4], op=Alu.mult)
                nc.vector.tensor_tensor(out=t0, in0=t0, in1=an[:, :, 0:2], op=Alu.add)
                t1 = pool.tile([128, PA, 2], f32, tag=f"t1{b}")
                nc.vector.tensor_scalar(t1, wh, 0.5, None, Alu.mult)
                nc.vector.tensor_tensor(out=tabt[:, :, 0:2], in0=t0, in1=t1,
                                        op=Alu.subtract)
                nc.vector.tensor_tensor(out=tabt[:, :, 2:4], in0=tabt[:, :, 0:2],
                                        in1=wh, op=Alu.add)
                t2 = pool.tile([128, PA, 2], f32, tag=f"t2{b}")
                nc.vector.tensor_tensor(out=t2, in0=tabt[:, :, 2:4],
                                        in1=tabt[:, :, 0:2], op=Alu.subtract)
                nc.vector.tensor_tensor(out=tabt[:, :, 4:5], in0=t2[:, :, 0:1],
                                        in1=t2[:, :, 1:2], op=Alu.mult)
                nc.vector.memset(tabt[:, :, 5:8], 0.0)
                nc.sync.dma_start(
                    out=tab_out[b, :, :].rearrange("(p k) f -> p k f", p=128),
                    in_=tabt)
    nc.compile()
    names = dict(confT=confT.name, locd=locd.name, anch=anch.name,
                 pi=pi_out.name, tab=tab_out.name)
    return nc, names


def _build_launch2(steps=K):
    nc = bacc.Bacc(None, target_bir_lowering=False)
    with tile.TileContext(nc) as tc:
        with tc.tile_pool(name="dram", bufs=1, space="DRAM") as dram, \
             tc.tile_pool(name="sb", bufs=1) as pool:
            g_in = dram.tile([BPC, C, 5, W], f32, kind="ExternalInput")
            pv_in = dram.tile([BPC, C, W], f32, kind="ExternalInput")
            rows_out = dram.tile([BPC, C, K, 8], f32, kind="ExternalOutput")

            iot = pool.tile([C, W], f32)
            nc.gpsimd.iota(iot, pattern=[[1, W]], base=0, channel_multiplier=0,
                           allow_small_or_imprecise_dtypes=True)
            negC = pool.tile([C, W], f32)
            nc.vector.memset(negC, NEG)

            V = nc.vector
            stt = V.scalar_tensor_tensor
            ctx = []
            for b in range(BPC):
                G = pool.tile([C, 5, W], f32, tag=f"G{b}")
                nc.sync.dma_start(out=G, in_=g_in[b, :, :, :])
                pv = pool.tile([C, W], f32, tag=f"pv{b}")
                nc.sync.dma_start(out=pv, in_=pv_in[b, :, :])
                # s = where(conf > x*, conf, NEG) on exact raw conf
                cmp = pool.tile([C, W], u32, tag=f"cmp{b}")
                s = pool.tile([C, W], f32, tag=f"s{b}")
                V.tensor_scalar(cmp, pv, XSTAR, None, Alu.is_gt)
                V.select(out=s, mask=cmp, on_true=pv, on_false=negC)
                d = dict(G=G, s=s)
                for nm, shp, dt in (("i8", [C, 8], u32), ("jf", [C, 1], f32),
                                    ("scr", [C, W], f32), ("wh2", [C, 2], f32),
                                    ("m1", [C, W], f32), ("wx", [C, W], f32),
                                    ("wy", [C, W], f32), ("cx", [C, W], f32),
                                    ("int", [C, W], f32), ("un", [C, W], f32),
                                    ("ddm", [C, W], f32),
                                    ("outb", [C, K, 8], f32)):
                    d[nm] = pool.tile(shp, dt, tag=f"{nm}{b}", name=f"{nm}{b}")
                ctx.append(d)

            # the two batches' chains are independent: interleave their step
            # bodies so the DVE pipeline always has a ready instruction
            for t in range(steps):
                for d in ctx:
                    G, s, outb = d["G"], d["s"], d["outb"]
                    i8, jf, scr = d["i8"], d["jf"], d["scr"]
                    # max writes the row head directly: field 0 = score
                    V.max(out=outb[:, t, 0:8], in_=s)
                    V.max_index(out=i8, in_max=outb[:, t, 0:8], in_values=s)
                    V.tensor_copy(jf, i8[:, 0:1])
                for d in ctx:
                    G, outb, jf, scr = d["G"], d["outb"], d["jf"], d["scr"]
                    # box extraction: coords -> outb[:, t, 1:5] (one pass/field)
                    for f in range(4):
                        stt(out=scr, in0=iot, scalar=jf[:, 0:1], in1=G[:, f, :],
                            op0=Alu.is_equal, op1=Alu.mult,
                            accum_out=outb[:, t, f + 1:f + 2])
                for d in ctx:
                    G, outb, wh2, m1 = d["G"], d["outb"], d["wh2"], d["m1"]
                    # selected area from corners (reference fp-op order)
                    V.tensor_tensor(out=wh2, in0=outb[:, t, 3:5],
                                    in1=outb[:, t, 1:3], op=Alu.subtract)
                    V.tensor_tensor(out=outb[:, t, 5:6], in0=wh2[:, 0:1],
                                    in1=wh2[:, 1:2], op=Alu.mult)
                    # IoU: w = min(Gx2,X2)-max(Gx1,X1), same for y
                    V.tensor_scalar(m1, G[:, 0, :], outb[:, t, 1:2],
                                    None, Alu.max)
                for d in ctx:
                    G, outb, m1, wx = d["G"], d["outb"], d["m1"], d["wx"]
                    stt(out=wx, in0=G[:, 2, :], scalar=outb[:, t, 3:4], in1=m1,
                        op0=Alu.min, op1=Alu.subtract)
                    V.tensor_scalar(m1, G[:, 1, :], outb[:, t, 2:3],
                                    None, Alu.max)
                for d in ctx:
                    G, outb, m1, wy, cx = d["G"], d["outb"], d["m1"], d["wy"], d["cx"]
                    stt(out=wy, in0=G[:, 3, :], scalar=outb[:, t, 4:5], in1=m1,
                        op0=Alu.min, op1=Alu.subtract)
                    V.tensor_scalar(cx, d["wx"], 0.0, None, Alu.max)
                for d in ctx:
                    stt(out=d["int"], in0=d["wy"], scalar=0.0, in1=d["cx"],
                        op0=Alu.max, op1=Alu.mult)
                for d in ctx:
                    # union = (a_j + a_sel) - inter
                    stt(out=d["un"], in0=d["G"][:, 4, :],
                        scalar=d["outb"][:, t, 5:6], in1=d["int"],
                        op0=Alu.add, op1=Alu.subtract)
                for d in ctx:
                    # suppress iff inter > 0.5*union (no boundary flips)
                    stt(out=d["ddm"], in0=d["un"], scalar=0.5, in1=d["int"],
                        op0=Alu.mult, op1=Alu.is_lt)
                for d in ctx:
                    # s -= 1e30 * ddm  (== where(ddm, NEG, s))
                    stt(out=d["s"], in0=d["ddm"], scalar=NEG, in1=d["s"],
                        op0=Alu.mult, op1=Alu.add)

            for b in range(BPC):
                outb = ctx[b]["outb"]
                # epilogue: km mask, sigmoid on winning raw scores
                km = pool.tile([C, K], f32, tag=f"km{b}")
                xs = pool.tile([C, K], f32, tag=f"xs{b}")
                V.tensor_scalar(km, outb[:, :, 0], -1e29, None, Alu.is_gt)
                V.tensor_scalar(xs, outb[:, :, 0], -30.0, None, Alu.max)
                V.tensor_scalar(xs, xs, -1.0, None, Alu.mult)
                e = _exp_chain(nc, pool, xs[:, :], C, K, f"se{b}")
                den = pool.tile([C, K], f32, tag=f"den{b}")
                V.tensor_scalar(den, e, 1.0, None, Alu.add)
                sg = pool.tile([C, K], f32, tag=f"sg{b}")
                V.reciprocal(sg, den)
                V.tensor_copy(outb[:, :, 0], sg)
                # zero dead rows: fields 0:6 *= km
                km_ap = km[:, :]
                km_b = bass.AP(km_ap.tensor, km_ap.offset,
                               [list(km_ap.ap[0]), list(km_ap.ap[1]), [0, 6]])
                V.tensor_tensor(out=outb[:, :, 0:6], in0=outb[:, :, 0:6],
                                in1=km_b, op=Alu.mult)
                nc.sync.dma_start(out=rows_out[b, :, :, :], in_=outb)
    nc.compile()
    names = dict(g=g_in.name, pv=pv_in.name, rows=rows_out.name)
    return nc, names


_cache = {}


def _prep_launch2_inputs(r1, n1, confT, loc=None):
    """Host gather: exact conf + box-table fields at pool indices."""
    in_maps2 = []
    for c in range(NCORES):
        res = r1.results[c]
        pi, tab = res[n1["pi"]], res[n1["tab"]]
        G = np.empty((BPC, C, 5, W), np.float32)
        pv = np.empty((BPC, C, W), np.float32)
        for b in range(BPC):
            idx = pi[b].astype(np.int64)                 # [C, W]
            G[b] = tab[b][idx][..., :5].transpose(0, 2, 1)
            pv[b] = np.take_along_axis(confT[c * BPC + b], idx, axis=1)
        in_maps2.append({_cache["n2"]["g"]: np.ascontiguousarray(G),
                         _cache["n2"]["pv"]: pv})
    return in_maps2


def kernel(loc, conf, anchors):
    loc = np.ascontiguousarray(np.asarray(loc, np.float32))
    anchors = np.ascontiguousarray(np.asarray(anchors, np.float32))
    confT = np.ascontiguousarray(np.swapaxes(np.asarray(conf, np.float32), 1, 2))

    if "l1" not in _cache:
        _cache["l1"] = _build_launch1()
        _cache["l2"] = _build_launch2()
        _cache["n1"] = _cache["l1"][1]
        _cache["n2"] = _cache["l2"][1]
    nc1, n1 = _cache["l1"]
    nc2, n2 = _cache["l2"]

    in_maps = []
    for c in range(NCORES):
        sl = slice(c * BPC, (c + 1) * BPC)
        in_maps.append({n1["confT"]: confT[sl], n1["locd"]: loc[sl],
                        n1["anch"]: anchors})
    r1 = run_bass_kernel_spmd(nc1, in_maps, core_ids=list(range(NCORES)))

    in_maps2 = _prep_launch2_inputs(r1, n1, confT)
    r2 = run_bass_kernel_spmd(nc2, in_maps2, core_ids=list(range(NCORES)))

    out = np.empty((B, C, K, 5), np.float32)
    for c in range(NCORES):
        rows = r2.results[c][n2["rows"]]
        out[c * BPC:(c + 1) * BPC] = rows[..., :5]
    return out
